# revision 30
# baseline (speedup 1.0000x reference)
"""CRF loss (forward-algorithm partition function minus gold path score, batch mean)
on 8 Trainium2 NeuronCores, data-parallel over the batch dimension.

Layout / algorithm notes
------------------------
Per core shard: 512 batches = 4 groups x 128 batch-columns.
State tiles [128 part = (group, tag), 128 free = batch col] in bf16.

The partition function runs as a BIDIRECTIONAL exp-space scan meeting in the
middle, which halves the serial matmul->multiply dependency chain:

    fwd:  alpha_s = (Mblk^T  @ alpha_{s-1}) * xp_s      s = 1..255
    bwd:  beta_s  = (MblkT^T @ beta_{s+1})  * xp_s      s = 510..256
    Z_b  = sum_t alpha_255[t,b] * (M beta_256)[t,b]

with xp_s = exp(em_s - MU), Mblk = blockdiag(exp(T)), MblkT =
blockdiag(exp(T)^T).  MU = log(T)+1 cancels the mean per-step growth, so the
state drifts only O(sqrt(S)) nats per batch and needs NO rescaling inside
fp32/bf16 exponent range; logZ = log(Z_b) + S*MU exactly.

The two chains are independent, so the tensor engine interleaves fwd/bwd
matmuls while the vector engine interleaves the emission multiplies: the
per-step serial latency is paid only 256 times instead of 512.

The gold path score is pure integer indexing on tags (gathers + bincounts)
plus one emission gather; it is computed on host in float64 (the device keeps
the O(B*S*T^2) forward algorithm).
"""

import numpy as np
import ml_dtypes

B, S, T = 4096, 512, 32
NCORES = 8
BS = B // NCORES          # batches per core
G, BG = 4, 128            # groups x batch-columns (G*BG == BS)
P = 128
HALF = S // 2             # steps per chain
CS = [4, 12, 16] + [32] * 7   # chunk sizes (steps); small first chunks = fast ramp
CO = np.cumsum([0] + CS).tolist()   # chunk start offsets
NCH = len(CS)
MU = float(np.log(T) + 1.0)

BF16 = ml_dtypes.bfloat16

_GRAPH = None


def _build_graph():
    from concourse import bacc, mybir, tile

    f32 = mybir.dt.float32
    bf16 = mybir.dt.bfloat16
    Af = mybir.ActivationFunctionType
    Op = mybir.AluOpType
    AX = mybir.AxisListType.X

    nc = bacc.Bacc(
        "TRN2",
        target_bir_lowering=False,
        debug=False,
        enable_asserts=False,
        num_devices=NCORES,
    )

    em_scan = nc.dram_tensor("em_scan", [P, S * BG], bf16, kind="ExternalInput")
    mblks_in = nc.dram_tensor("mblks", [P, 2 * P], bf16, kind="ExternalInput")
    esee_in = nc.dram_tensor("esee", [P, 2], f32, kind="ExternalInput")  # exp(start), exp(end)
    bones4 = nc.dram_tensor("bones4", [P, G], bf16, kind="ExternalInput")
    out = nc.dram_tensor("out", [1, 1], f32, kind="ExternalOutput")

    em_ap = em_scan.ap()
    BWD0 = HALF * BG          # column offset of the backward stream

    with tile.TileContext(nc) as tc:
        with (
            tc.tile_pool(name="cpool", bufs=1) as cpool,
            tc.tile_pool(name="emp", bufs=3) as emp,
            tc.tile_pool(name="xpp", bufs=3) as xpp,
            tc.tile_pool(name="apool", bufs=3) as apool,
            tc.tile_pool(name="bpool", bufs=3) as bpool,
            tc.tile_pool(name="psf", bufs=3, space="PSUM") as psfp,
            tc.tile_pool(name="psb", bufs=3, space="PSUM") as psbp,
            tc.tile_pool(name="psx", bufs=1, space="PSUM") as psxp,
        ):
            # ---- constants on the scalar DGE queue; chunks own the sync queue ----
            esee_t = cpool.tile([P, 2], f32)
            nc.scalar.dma_start(out=esee_t[:], in_=esee_in.ap())
            es_t, ee_t = esee_t[:, 0:1], esee_t[:, 1:2]
            mblks_t = cpool.tile([P, 2 * P], bf16)
            nc.scalar.dma_start(out=mblks_t[:], in_=mblks_in.ap())
            mblk_t, mblkT_t = mblks_t[:, 0:P], mblks_t[:, P : 2 * P]

            # warm the Exp table while the first DMAs are in flight
            negmu_t = cpool.tile([P, 1], f32)
            nc.vector.memset(negmu_t[:], -MU)
            warm_t = cpool.tile([P, 1], f32)
            nc.scalar.activation(warm_t[:], negmu_t[:], Af.Exp)

            # ---- emission chunk stream ----
            # The host lays out each chunk as [fwd steps | bwd steps]
            # contiguously, so one DMA feeds both chains; exp runs in 8-step
            # slices alternating f/b so the first windows of the chunk
            # unblock early on both chains.
            def issue_chunk(c):
                lo, n = 2 * CO[c] * BG, CS[c] * BG
                em_t = emp.tile([P, 2 * n], bf16, name="em")
                nc.sync.dma_start(out=em_t[:], in_=em_ap[:, lo : lo + 2 * n])
                xp_t = xpp.tile([P, 2 * n], bf16, name="xp")
                sz = 8 if c < 3 else 32
                o = 0
                while o < CS[c]:
                    sl = min(sz, CS[c] - o)
                    a, b = o * BG, (o + sl) * BG
                    nc.scalar.activation(xp_t[:, a:b], em_t[:, a:b], Af.Exp, bias=negmu_t[:])
                    nc.scalar.activation(
                        xp_t[:, n + a : n + b], em_t[:, n + a : n + b], Af.Exp, bias=negmu_t[:]
                    )
                    o += sl
                return xp_t

            # window -> (chunk, offset-in-chunk) map
            w2c = []
            for ci, n in enumerate(CS):
                w2c += [(ci, so) for so in range(n)]

            xp_t = issue_chunk(0)
            pending = [issue_chunk(1), issue_chunk(2)]

            bones4_t = cpool.tile([P, G], bf16)
            nc.scalar.dma_start(out=bones4_t[:], in_=bones4.ap())
            onesG_t = cpool.tile([G, 1], f32)
            nc.vector.memset(onesG_t[:], 1.0)

            # ---- init both chains (window 0) ----
            boff = CS[0] * BG
            alpha = apool.tile([P, BG], bf16, tag="alpha", name="alpha")
            nc.vector.tensor_scalar_mul(alpha[:], xp_t[:, 0:BG], es_t)
            beta = bpool.tile([P, BG], bf16, tag="beta", name="beta")
            nc.vector.tensor_scalar_mul(beta[:], xp_t[:, boff : boff + BG], ee_t)

            # ---- main bidirectional scan: windows 1..HALF-1 ----
            for w in range(1, HALF):
                c, so = w2c[w]
                if so == 0:
                    xp_t = pending.pop(0)
                    boff = CS[c] * BG
                    if c + 2 < NCH:
                        pending.append(issue_chunk(c + 2))

                psf = psfp.tile([P, BG], f32, tag="psf", name="psf")
                nc.tensor.matmul(psf[:], lhsT=mblk_t, rhs=alpha[:], start=True, stop=True)
                psb = psbp.tile([P, BG], f32, tag="psb", name="psb")
                nc.tensor.matmul(psb[:], lhsT=mblkT_t, rhs=beta[:], start=True, stop=True)

                alpha_new = apool.tile([P, BG], bf16, tag="alpha", name="alpha")
                nc.vector.tensor_tensor(
                    alpha_new[:], psf[:], xp_t[:, so * BG : (so + 1) * BG], Op.mult
                )
                alpha = alpha_new
                beta_new = bpool.tile([P, BG], bf16, tag="beta", name="beta")
                nc.vector.tensor_tensor(
                    beta_new[:], psb[:], xp_t[:, boff + so * BG : boff + (so + 1) * BG], Op.mult
                )
                beta = beta_new

            # ---- junction: Z = sum_t alpha_255 * (M beta_256) ----
            psj = psfp.tile([P, BG], f32, tag="psf", name="psj")
            nc.tensor.matmul(psj[:], lhsT=mblkT_t, rhs=beta[:], start=True, stop=True)
            zt = apool.tile([P, BG], bf16, tag="alpha", name="zt")
            nc.vector.tensor_tensor(zt[:], psj[:], alpha[:], Op.mult)

            gs = psxp.tile([G, BG], f32, tag="gs", name="gs")
            nc.tensor.matmul(gs[:], lhsT=bones4_t[:], rhs=zt[:], start=True, stop=True)
            lngs_t = cpool.tile([G, BG], f32)
            nc.scalar.activation(lngs_t[:], gs[:], Af.Ln)
            colsum_t = cpool.tile([G, 1], f32)
            nc.vector.reduce_sum(colsum_t[:], lngs_t[:], axis=AX)

            fin = psxp.tile([1, 1], f32, tag="fin", name="fin")
            nc.tensor.matmul(fin[:], lhsT=onesG_t[:], rhs=colsum_t[:], start=True, stop=True)
            outsb = cpool.tile([1, 1], f32)
            nc.vector.tensor_copy(outsb[:], fin[:])
            nc.sync.dma_start(out=out.ap(), in_=outsb[:])

    nc.compile()
    return nc


def _get_graph():
    global _GRAPH
    if _GRAPH is None:
        _GRAPH = _build_graph()
    return _GRAPH


def _host_inputs(transitions, start_transitions, end_transitions):
    """Constant / parameter-layout tensors shared by all cores (already
    exponentiated so the device preamble is DMA-only)."""
    Tm = np.asarray(transitions, np.float32)
    sv = np.asarray(start_transitions, np.float32)
    ev = np.asarray(end_transitions, np.float32)

    Mexp = np.exp(Tm).astype(BF16)
    MexpT = np.exp(Tm.T).astype(BF16)
    mblks = np.zeros((P, 2 * P), BF16)
    for g in range(G):
        sl = slice(g * 32, (g + 1) * 32)
        mblks[sl, sl] = Mexp
        mblks[sl, P + g * 32 : P + (g + 1) * 32] = MexpT

    esee = np.stack(
        [np.exp(np.tile(sv, G)), np.exp(np.tile(ev, G))], axis=1
    ).astype(np.float32)

    k = np.arange(P)
    bones4 = (np.arange(G)[None, :] == (k[:, None] // 32)).astype(BF16)  # [P, G]

    return {
        "mblks": mblks,
        "esee": np.ascontiguousarray(esee),
        "bones4": np.ascontiguousarray(bones4),
    }


_CHUNK_STEP_ORDER = None


def _chunk_step_order():
    """Scan-axis permutation: per chunk, fwd steps then (reversed) bwd steps."""
    global _CHUNK_STEP_ORDER
    if _CHUNK_STEP_ORDER is None:
        order = []
        for c in range(NCH):
            lo, n = CO[c], CS[c]
            order += list(range(lo, lo + n))                    # fwd s
            order += list(range(HALF + lo, HALF + lo + n))      # bwd slot
        _CHUNK_STEP_ORDER = np.array(order)
    return _CHUNK_STEP_ORDER


def _shard_inputs(emissions, core):
    """Per-core scan-layout emissions: per chunk, fwd steps then reversed bwd
    steps, so one contiguous DMA feeds both chains."""
    bsl = slice(core * BS, (core + 1) * BS)
    em4 = np.asarray(emissions[bsl], np.float32).reshape(G, BG, S, T)
    emf = em4[:, :, :HALF, :]                       # s = 0..255
    emb = em4[:, :, HALF:, :][:, :, ::-1, :]        # s = 511..256
    both = np.concatenate([emf, emb], axis=2)       # [G, BG, S, T]
    both = both[:, :, _chunk_step_order(), :]       # chunk-interleaved
    em_scan = both.transpose(0, 3, 2, 1).reshape(P, S * BG).astype(BF16)
    return {"em_scan": np.ascontiguousarray(em_scan)}


def _gold_host(emissions, tags, transitions, start_transitions, end_transitions):
    """Gold path score summed over the batch in float64 (pure tag indexing
    plus one emission gather)."""
    tg = np.asarray(tags).astype(np.int64)
    em = np.asarray(emissions)
    emit_sum = np.take_along_axis(em, tg[:, :, None], axis=2)[..., 0].sum(
        dtype=np.float64
    )
    trans_sum = np.asarray(transitions)[tg[:, :-1], tg[:, 1:]].sum(dtype=np.float64)
    start_sum = np.asarray(start_transitions)[tg[:, 0]].sum(dtype=np.float64)
    end_sum = np.asarray(end_transitions)[tg[:, -1]].sum(dtype=np.float64)
    return emit_sum + trans_sum + start_sum + end_sum


def _numpy_reference(emissions, tags, mask, transitions, start_transitions, end_transitions):
    """Slow numpy fallback, only used if mask is not all ones."""
    em = np.asarray(emissions, np.float64)
    tg = np.asarray(tags).astype(np.int64)
    mk = np.asarray(mask).astype(bool)
    Tm = np.asarray(transitions, np.float64)
    sv = np.asarray(start_transitions, np.float64)
    ev = np.asarray(end_transitions, np.float64)
    Bn, Sn, Tn = em.shape

    t0 = tg[:, 0]
    score = sv[t0] + np.take_along_axis(em[:, 0], t0[:, None], axis=1)[:, 0]
    maskf = mk[:, 1:].astype(np.float64)
    trans_sc = Tm[tg[:, :-1], tg[:, 1:]]
    emit_sc = np.take_along_axis(em[:, 1:], tg[:, 1:, None], axis=2)[..., 0]
    gold = score + ((trans_sc + emit_sc) * maskf).sum(axis=1)
    last_idx = mk.sum(axis=1).astype(np.int64) - 1
    last_tags = np.take_along_axis(tg, last_idx[:, None], axis=1)[:, 0]
    gold = gold + ev[last_tags]

    sc = sv[None, :] + em[:, 0]
    for s in range(1, Sn):
        nxt = sc[:, :, None] + Tm[None] + em[:, s][:, None, :]
        m = nxt.max(axis=1)
        nxt = m + np.log(np.exp(nxt - m[:, None, :]).sum(axis=1))
        sc = np.where(mk[:, s][:, None], nxt, sc)
    sc = sc + ev[None, :]
    m = sc.max(axis=1)
    fwd = m + np.log(np.exp(sc - m[:, None]).sum(axis=1))
    return np.array((fwd - gold).mean(), np.float32)


def kernel(emissions, tags, mask, transitions, start_transitions, end_transitions,
           _want_results=False, _trace=False):
    emissions = np.asarray(emissions)
    tags = np.asarray(tags)
    mask = np.asarray(mask)

    if not mask.all():
        return _numpy_reference(
            emissions, tags, mask, transitions, start_transitions, end_transitions
        )

    from concourse.bass_utils import run_bass_kernel_spmd

    nc = _get_graph()
    shared = _host_inputs(transitions, start_transitions, end_transitions)
    in_maps = []
    for c in range(NCORES):
        m = dict(shared)
        m.update(_shard_inputs(emissions, c))
        in_maps.append(m)

    res = run_bass_kernel_spmd(nc, in_maps, list(range(NCORES)), trace=_trace)

    gold = _gold_host(emissions, tags, transitions, start_transitions, end_transitions)
    tot_fwd = 0.0
    for c in range(NCORES):
        tot_fwd += float(np.asarray(res.results[c]["out"], np.float64)[0, 0])
    tot_fwd += B * S * MU
    loss = (tot_fwd - gold) / B
    if _want_results:
        return np.array(loss, np.float32), res
    return np.array(loss, np.float32)


# revision 32
# speedup vs baseline: 1.0076x; 1.0076x over previous
"""CRF loss (forward-algorithm partition function minus gold path score, batch mean)
on 8 Trainium2 NeuronCores, data-parallel over the batch dimension.

Layout / algorithm notes
------------------------
Per core shard: 512 batches = 4 groups x 128 batch-columns.
State tiles [128 part = (group, tag), 128 free = batch col] in bf16.

The partition function runs as a BIDIRECTIONAL exp-space scan meeting in the
middle, which halves the serial matmul->multiply dependency chain:

    fwd:  alpha_s = (Mblk^T  @ alpha_{s-1}) * xp_s      s = 1..255
    bwd:  beta_s  = (MblkT^T @ beta_{s+1})  * xp_s      s = 510..256
    Z_b  = sum_t alpha_255[t,b] * (M beta_256)[t,b]

with xp_s = exp(em_s - MU), Mblk = blockdiag(exp(T)), MblkT =
blockdiag(exp(T)^T).  MU = log(T)+1 cancels the mean per-step growth, so the
state drifts only O(sqrt(S)) nats per batch and needs NO rescaling inside
fp32/bf16 exponent range; logZ = log(Z_b) + S*MU exactly.

The two chains are independent, so the tensor engine interleaves fwd/bwd
matmuls while the vector engine interleaves the emission multiplies: the
per-step serial latency is paid only 256 times instead of 512.

The gold path score is pure integer indexing on tags (gathers + bincounts)
plus one emission gather; it is computed on host in float64 (the device keeps
the O(B*S*T^2) forward algorithm).
"""

import numpy as np
import ml_dtypes

B, S, T = 4096, 512, 32
NCORES = 8
BS = B // NCORES          # batches per core
G, BG = 4, 128            # groups x batch-columns (G*BG == BS)
P = 128
HALF = S // 2             # steps per chain
CS = [4, 12, 16] + [32] * 7   # chunk sizes (steps); small first chunks = fast ramp
CO = np.cumsum([0] + CS).tolist()   # chunk start offsets
NCH = len(CS)
MU = float(np.log(T) + 1.0)

BF16 = ml_dtypes.bfloat16

_GRAPH = None


def _build_graph():
    from concourse import bacc, mybir, tile

    f32 = mybir.dt.float32
    bf16 = mybir.dt.bfloat16
    Af = mybir.ActivationFunctionType
    Op = mybir.AluOpType
    AX = mybir.AxisListType.X

    nc = bacc.Bacc(
        "TRN2",
        target_bir_lowering=False,
        debug=False,
        enable_asserts=False,
        num_devices=NCORES,
    )

    em_scan = nc.dram_tensor("em_scan", [P, S * BG], bf16, kind="ExternalInput")
    mblks_in = nc.dram_tensor("mblks", [P, 2 * P], bf16, kind="ExternalInput")
    esee_in = nc.dram_tensor("esee", [P, 2], f32, kind="ExternalInput")  # exp(start), exp(end)
    bones4 = nc.dram_tensor("bones4", [P, G], bf16, kind="ExternalInput")
    out = nc.dram_tensor("out", [1, 1], f32, kind="ExternalOutput")

    em_ap = em_scan.ap()
    BWD0 = HALF * BG          # column offset of the backward stream

    with tile.TileContext(nc) as tc:
        with (
            tc.tile_pool(name="cpool", bufs=1) as cpool,
            tc.tile_pool(name="emp", bufs=3) as emp,
            tc.tile_pool(name="xpp", bufs=3) as xpp,
            tc.tile_pool(name="apool", bufs=3) as apool,
            tc.tile_pool(name="bpool", bufs=3) as bpool,
            tc.tile_pool(name="psf", bufs=3, space="PSUM") as psfp,
            tc.tile_pool(name="psb", bufs=3, space="PSUM") as psbp,
            tc.tile_pool(name="psx", bufs=1, space="PSUM") as psxp,
        ):
            # ---- constants first: tiny DMAs, land before chunk 0 finishes ----
            esee_t = cpool.tile([P, 2], f32)
            nc.sync.dma_start(out=esee_t[:], in_=esee_in.ap())
            es_t, ee_t = esee_t[:, 0:1], esee_t[:, 1:2]
            mblks_t = cpool.tile([P, 2 * P], bf16)
            nc.sync.dma_start(out=mblks_t[:], in_=mblks_in.ap())
            mblk_t, mblkT_t = mblks_t[:, 0:P], mblks_t[:, P : 2 * P]

            # warm the Exp table while the first DMAs are in flight
            negmu_t = cpool.tile([P, 1], f32)
            nc.vector.memset(negmu_t[:], -MU)
            warm_t = cpool.tile([P, 1], f32)
            nc.scalar.activation(warm_t[:], negmu_t[:], Af.Exp)

            # ---- emission chunk stream ----
            # The host lays out each chunk as [fwd steps | bwd steps]
            # contiguously, so one DMA feeds both chains; exp runs in 8-step
            # slices alternating f/b so the first windows of the chunk
            # unblock early on both chains.
            def issue_chunk(c):
                lo, n = 2 * CO[c] * BG, CS[c] * BG
                em_t = emp.tile([P, 2 * n], bf16, name="em")
                nc.sync.dma_start(out=em_t[:], in_=em_ap[:, lo : lo + 2 * n])
                xp_t = xpp.tile([P, 2 * n], bf16, name="xp")
                sz = 8 if c < 3 else 32
                o = 0
                while o < CS[c]:
                    sl = min(sz, CS[c] - o)
                    a, b = o * BG, (o + sl) * BG
                    nc.scalar.activation(xp_t[:, a:b], em_t[:, a:b], Af.Exp, bias=negmu_t[:])
                    nc.scalar.activation(
                        xp_t[:, n + a : n + b], em_t[:, n + a : n + b], Af.Exp, bias=negmu_t[:]
                    )
                    o += sl
                return xp_t

            # window -> (chunk, offset-in-chunk) map
            w2c = []
            for ci, n in enumerate(CS):
                w2c += [(ci, so) for so in range(n)]

            xp_t = issue_chunk(0)
            pending = [issue_chunk(1)]

            bones4_t = cpool.tile([P, G], bf16)
            nc.sync.dma_start(out=bones4_t[:], in_=bones4.ap())
            onesG_t = cpool.tile([G, 1], f32)
            nc.vector.memset(onesG_t[:], 1.0)

            pending.append(issue_chunk(2))

            # ---- init both chains (window 0) ----
            boff = CS[0] * BG
            alpha = apool.tile([P, BG], bf16, tag="alpha", name="alpha")
            nc.vector.tensor_scalar_mul(alpha[:], xp_t[:, 0:BG], es_t)
            beta = bpool.tile([P, BG], bf16, tag="beta", name="beta")
            nc.vector.tensor_scalar_mul(beta[:], xp_t[:, boff : boff + BG], ee_t)

            # ---- main bidirectional scan: windows 1..HALF-1 ----
            for w in range(1, HALF):
                c, so = w2c[w]
                if so == 0:
                    xp_t = pending.pop(0)
                    boff = CS[c] * BG
                    if c + 2 < NCH:
                        pending.append(issue_chunk(c + 2))

                psf = psfp.tile([P, BG], f32, tag="psf", name="psf")
                nc.tensor.matmul(psf[:], lhsT=mblk_t, rhs=alpha[:], start=True, stop=True)
                psb = psbp.tile([P, BG], f32, tag="psb", name="psb")
                nc.tensor.matmul(psb[:], lhsT=mblkT_t, rhs=beta[:], start=True, stop=True)

                alpha_new = apool.tile([P, BG], bf16, tag="alpha", name="alpha")
                nc.vector.tensor_tensor(
                    alpha_new[:], psf[:], xp_t[:, so * BG : (so + 1) * BG], Op.mult
                )
                alpha = alpha_new
                beta_new = bpool.tile([P, BG], bf16, tag="beta", name="beta")
                nc.vector.tensor_tensor(
                    beta_new[:], psb[:], xp_t[:, boff + so * BG : boff + (so + 1) * BG], Op.mult
                )
                beta = beta_new

            # ---- junction: Z = sum_t alpha_255 * (M beta_256) ----
            psj = psfp.tile([P, BG], f32, tag="psf", name="psj")
            nc.tensor.matmul(psj[:], lhsT=mblkT_t, rhs=beta[:], start=True, stop=True)
            zt = apool.tile([P, BG], bf16, tag="alpha", name="zt")
            nc.vector.tensor_tensor(zt[:], psj[:], alpha[:], Op.mult)

            gs = psxp.tile([G, BG], f32, tag="gs", name="gs")
            nc.tensor.matmul(gs[:], lhsT=bones4_t[:], rhs=zt[:], start=True, stop=True)
            lngs_t = cpool.tile([G, BG], f32)
            nc.scalar.activation(lngs_t[:], gs[:], Af.Ln)
            colsum_t = cpool.tile([G, 1], f32)
            nc.vector.reduce_sum(colsum_t[:], lngs_t[:], axis=AX)

            fin = psxp.tile([1, 1], f32, tag="fin", name="fin")
            nc.tensor.matmul(fin[:], lhsT=onesG_t[:], rhs=colsum_t[:], start=True, stop=True)
            outsb = cpool.tile([1, 1], f32)
            nc.vector.tensor_copy(outsb[:], fin[:])
            nc.sync.dma_start(out=out.ap(), in_=outsb[:])

    nc.compile()
    return nc


def _get_graph():
    global _GRAPH
    if _GRAPH is None:
        _GRAPH = _build_graph()
    return _GRAPH


def _host_inputs(transitions, start_transitions, end_transitions):
    """Constant / parameter-layout tensors shared by all cores (already
    exponentiated so the device preamble is DMA-only)."""
    Tm = np.asarray(transitions, np.float32)
    sv = np.asarray(start_transitions, np.float32)
    ev = np.asarray(end_transitions, np.float32)

    Mexp = np.exp(Tm).astype(BF16)
    MexpT = np.exp(Tm.T).astype(BF16)
    mblks = np.zeros((P, 2 * P), BF16)
    for g in range(G):
        sl = slice(g * 32, (g + 1) * 32)
        mblks[sl, sl] = Mexp
        mblks[sl, P + g * 32 : P + (g + 1) * 32] = MexpT

    esee = np.stack(
        [np.exp(np.tile(sv, G)), np.exp(np.tile(ev, G))], axis=1
    ).astype(np.float32)

    k = np.arange(P)
    bones4 = (np.arange(G)[None, :] == (k[:, None] // 32)).astype(BF16)  # [P, G]

    return {
        "mblks": mblks,
        "esee": np.ascontiguousarray(esee),
        "bones4": np.ascontiguousarray(bones4),
    }


_CHUNK_STEP_ORDER = None


def _chunk_step_order():
    """Scan-axis permutation: per chunk, fwd steps then (reversed) bwd steps."""
    global _CHUNK_STEP_ORDER
    if _CHUNK_STEP_ORDER is None:
        order = []
        for c in range(NCH):
            lo, n = CO[c], CS[c]
            order += list(range(lo, lo + n))                    # fwd s
            order += list(range(HALF + lo, HALF + lo + n))      # bwd slot
        _CHUNK_STEP_ORDER = np.array(order)
    return _CHUNK_STEP_ORDER


def _shard_inputs(emissions, core):
    """Per-core scan-layout emissions: per chunk, fwd steps then reversed bwd
    steps, so one contiguous DMA feeds both chains."""
    bsl = slice(core * BS, (core + 1) * BS)
    em4 = np.asarray(emissions[bsl], np.float32).reshape(G, BG, S, T)
    emf = em4[:, :, :HALF, :]                       # s = 0..255
    emb = em4[:, :, HALF:, :][:, :, ::-1, :]        # s = 511..256
    both = np.concatenate([emf, emb], axis=2)       # [G, BG, S, T]
    both = both[:, :, _chunk_step_order(), :]       # chunk-interleaved
    em_scan = both.transpose(0, 3, 2, 1).reshape(P, S * BG).astype(BF16)
    return {"em_scan": np.ascontiguousarray(em_scan)}


def _gold_host(emissions, tags, transitions, start_transitions, end_transitions):
    """Gold path score summed over the batch in float64 (pure tag indexing
    plus one emission gather)."""
    tg = np.asarray(tags).astype(np.int64)
    em = np.asarray(emissions)
    emit_sum = np.take_along_axis(em, tg[:, :, None], axis=2)[..., 0].sum(
        dtype=np.float64
    )
    trans_sum = np.asarray(transitions)[tg[:, :-1], tg[:, 1:]].sum(dtype=np.float64)
    start_sum = np.asarray(start_transitions)[tg[:, 0]].sum(dtype=np.float64)
    end_sum = np.asarray(end_transitions)[tg[:, -1]].sum(dtype=np.float64)
    return emit_sum + trans_sum + start_sum + end_sum


def _numpy_reference(emissions, tags, mask, transitions, start_transitions, end_transitions):
    """Slow numpy fallback, only used if mask is not all ones."""
    em = np.asarray(emissions, np.float64)
    tg = np.asarray(tags).astype(np.int64)
    mk = np.asarray(mask).astype(bool)
    Tm = np.asarray(transitions, np.float64)
    sv = np.asarray(start_transitions, np.float64)
    ev = np.asarray(end_transitions, np.float64)
    Bn, Sn, Tn = em.shape

    t0 = tg[:, 0]
    score = sv[t0] + np.take_along_axis(em[:, 0], t0[:, None], axis=1)[:, 0]
    maskf = mk[:, 1:].astype(np.float64)
    trans_sc = Tm[tg[:, :-1], tg[:, 1:]]
    emit_sc = np.take_along_axis(em[:, 1:], tg[:, 1:, None], axis=2)[..., 0]
    gold = score + ((trans_sc + emit_sc) * maskf).sum(axis=1)
    last_idx = mk.sum(axis=1).astype(np.int64) - 1
    last_tags = np.take_along_axis(tg, last_idx[:, None], axis=1)[:, 0]
    gold = gold + ev[last_tags]

    sc = sv[None, :] + em[:, 0]
    for s in range(1, Sn):
        nxt = sc[:, :, None] + Tm[None] + em[:, s][:, None, :]
        m = nxt.max(axis=1)
        nxt = m + np.log(np.exp(nxt - m[:, None, :]).sum(axis=1))
        sc = np.where(mk[:, s][:, None], nxt, sc)
    sc = sc + ev[None, :]
    m = sc.max(axis=1)
    fwd = m + np.log(np.exp(sc - m[:, None]).sum(axis=1))
    return np.array((fwd - gold).mean(), np.float32)


def kernel(emissions, tags, mask, transitions, start_transitions, end_transitions,
           _want_results=False, _trace=False):
    emissions = np.asarray(emissions)
    tags = np.asarray(tags)
    mask = np.asarray(mask)

    if not mask.all():
        return _numpy_reference(
            emissions, tags, mask, transitions, start_transitions, end_transitions
        )

    from concourse.bass_utils import run_bass_kernel_spmd

    nc = _get_graph()
    shared = _host_inputs(transitions, start_transitions, end_transitions)
    in_maps = []
    for c in range(NCORES):
        m = dict(shared)
        m.update(_shard_inputs(emissions, c))
        in_maps.append(m)

    res = run_bass_kernel_spmd(nc, in_maps, list(range(NCORES)), trace=_trace)

    gold = _gold_host(emissions, tags, transitions, start_transitions, end_transitions)
    tot_fwd = 0.0
    for c in range(NCORES):
        tot_fwd += float(np.asarray(res.results[c]["out"], np.float64)[0, 0])
    tot_fwd += B * S * MU
    loss = (tot_fwd - gold) / B
    if _want_results:
        return np.array(loss, np.float32), res
    return np.array(loss, np.float32)


# revision 33
# speedup vs baseline: 1.0109x; 1.0032x over previous
"""CRF loss (forward-algorithm partition function minus gold path score, batch mean)
on 8 Trainium2 NeuronCores, data-parallel over the batch dimension.

Layout / algorithm notes
------------------------
Per core shard: 512 batches = 4 groups x 128 batch-columns.
State tiles [128 part = (group, tag), 128 free = batch col] in bf16.

The partition function runs as a BIDIRECTIONAL exp-space scan meeting in the
middle, which halves the serial matmul->multiply dependency chain:

    fwd:  alpha_s = (Mblk^T  @ alpha_{s-1}) * xp_s      s = 1..255
    bwd:  beta_s  = (MblkT^T @ beta_{s+1})  * xp_s      s = 510..256
    Z_b  = sum_t alpha_255[t,b] * (M beta_256)[t,b]

with xp_s = exp(em_s - MU), Mblk = blockdiag(exp(T)), MblkT =
blockdiag(exp(T)^T).  MU = log(T)+1 cancels the mean per-step growth, so the
state drifts only O(sqrt(S)) nats per batch and needs NO rescaling inside
fp32/bf16 exponent range; logZ = log(Z_b) + S*MU exactly.

The two chains are independent, so the tensor engine interleaves fwd/bwd
matmuls while the vector engine interleaves the emission multiplies: the
per-step serial latency is paid only 256 times instead of 512.

The gold path score is pure integer indexing on tags (gathers + bincounts)
plus one emission gather; it is computed on host in float64 (the device keeps
the O(B*S*T^2) forward algorithm).
"""

import numpy as np
import ml_dtypes

B, S, T = 4096, 512, 32
NCORES = 8
BS = B // NCORES          # batches per core
G, BG = 4, 128            # groups x batch-columns (G*BG == BS)
P = 128
HALF = S // 2             # steps per chain
CS = [4, 12, 16] + [32] * 7   # chunk sizes (steps); small first chunks = fast ramp
CO = np.cumsum([0] + CS).tolist()   # chunk start offsets
NCH = len(CS)
MU = float(np.log(T) + 1.0)

BF16 = ml_dtypes.bfloat16

_GRAPH = None


def _build_graph():
    from concourse import bacc, mybir, tile

    f32 = mybir.dt.float32
    bf16 = mybir.dt.bfloat16
    Af = mybir.ActivationFunctionType
    Op = mybir.AluOpType
    AX = mybir.AxisListType.X

    nc = bacc.Bacc(
        "TRN2",
        target_bir_lowering=False,
        debug=False,
        enable_asserts=False,
        num_devices=NCORES,
    )

    em_scan = nc.dram_tensor("em_scan", [P, S * BG], bf16, kind="ExternalInput")
    mblks_in = nc.dram_tensor("mblks", [P, 2 * P], bf16, kind="ExternalInput")
    esee_in = nc.dram_tensor("esee", [P, 2], f32, kind="ExternalInput")  # exp(start), exp(end)
    bones4 = nc.dram_tensor("bones4", [P, G], bf16, kind="ExternalInput")
    out = nc.dram_tensor("out", [1, 1], f32, kind="ExternalOutput")

    em_ap = em_scan.ap()

    with tile.TileContext(nc) as tc:
        with (
            tc.tile_pool(name="cpool", bufs=1) as cpool,
            tc.tile_pool(name="emp", bufs=3) as emp,
            tc.tile_pool(name="xpp", bufs=3) as xpp,
            tc.tile_pool(name="apool", bufs=3) as apool,
            tc.tile_pool(name="bpool", bufs=3) as bpool,
            tc.tile_pool(name="psf", bufs=3, space="PSUM") as psfp,
            tc.tile_pool(name="psb", bufs=3, space="PSUM") as psbp,
            tc.tile_pool(name="psx", bufs=1, space="PSUM") as psxp,
        ):
            # ---- constants first: tiny DMAs, land before chunk 0 finishes ----
            esee_t = cpool.tile([P, 2], f32)
            nc.sync.dma_start(out=esee_t[:], in_=esee_in.ap())
            es_t, ee_t = esee_t[:, 0:1], esee_t[:, 1:2]
            mblks_t = cpool.tile([P, 2 * P], bf16)
            nc.sync.dma_start(out=mblks_t[:], in_=mblks_in.ap())
            mblk_t, mblkT_t = mblks_t[:, 0:P], mblks_t[:, P : 2 * P]

            # warm the Exp table while the first DMAs are in flight
            negmu_t = cpool.tile([P, 1], f32)
            nc.vector.memset(negmu_t[:], -MU)
            warm_t = cpool.tile([P, 1], f32)
            nc.scalar.activation(warm_t[:], negmu_t[:], Af.Exp)

            # ---- emission chunk stream ----
            # The host lays out each chunk as [fwd steps | bwd steps]
            # contiguously, so one DMA feeds both chains; exp runs in 8-step
            # slices alternating f/b so the first windows of the chunk
            # unblock early on both chains.
            def issue_chunk(c):
                lo, n = 2 * CO[c] * BG, CS[c] * BG
                em_t = emp.tile([P, 2 * n], bf16, name="em")
                nc.sync.dma_start(out=em_t[:], in_=em_ap[:, lo : lo + 2 * n])
                xp_t = xpp.tile([P, 2 * n], bf16, name="xp")
                sz = 8 if c < 3 else 32
                o = 0
                while o < CS[c]:
                    sl = min(sz, CS[c] - o)
                    a, b = o * BG, (o + sl) * BG
                    nc.scalar.activation(xp_t[:, a:b], em_t[:, a:b], Af.Exp, bias=negmu_t[:])
                    nc.scalar.activation(
                        xp_t[:, n + a : n + b], em_t[:, n + a : n + b], Af.Exp, bias=negmu_t[:]
                    )
                    o += sl
                return xp_t

            # window -> (chunk, offset-in-chunk) map
            w2c = []
            for ci, n in enumerate(CS):
                w2c += [(ci, so) for so in range(n)]

            xp_t = issue_chunk(0)
            pending = [issue_chunk(1)]

            bones4_t = cpool.tile([P, G], bf16)
            nc.sync.dma_start(out=bones4_t[:], in_=bones4.ap())
            onesG_t = cpool.tile([G, 1], f32)
            nc.vector.memset(onesG_t[:], 1.0)

            pending.append(issue_chunk(2))

            # ---- init both chains (window 0) ----
            boff = CS[0] * BG
            alpha = apool.tile([P, BG], bf16, tag="alpha", name="alpha")
            nc.vector.tensor_scalar_mul(alpha[:], xp_t[:, 0:BG], es_t)
            beta = bpool.tile([P, BG], bf16, tag="beta", name="beta")
            nc.vector.tensor_scalar_mul(beta[:], xp_t[:, boff : boff + BG], ee_t)

            # ---- main bidirectional scan: windows 1..HALF-1 ----
            for w in range(1, HALF):
                c, so = w2c[w]
                if so == 0:
                    xp_t = pending.pop(0)
                    boff = CS[c] * BG
                    if c + 2 < NCH:
                        pending.append(issue_chunk(c + 2))

                psf = psfp.tile([P, BG], f32, tag="psf", name="psf")
                nc.tensor.matmul(psf[:], lhsT=mblk_t, rhs=alpha[:], start=True, stop=True)
                psb = psbp.tile([P, BG], f32, tag="psb", name="psb")
                nc.tensor.matmul(psb[:], lhsT=mblkT_t, rhs=beta[:], start=True, stop=True)

                alpha_new = apool.tile([P, BG], bf16, tag="alpha", name="alpha")
                nc.vector.tensor_tensor(
                    alpha_new[:], psf[:], xp_t[:, so * BG : (so + 1) * BG], Op.mult
                )
                alpha = alpha_new
                beta_new = bpool.tile([P, BG], bf16, tag="beta", name="beta")
                nc.vector.tensor_tensor(
                    beta_new[:], psb[:], xp_t[:, boff + so * BG : boff + (so + 1) * BG], Op.mult
                )
                beta = beta_new

            # ---- junction: Z = sum_t alpha_255 * (M beta_256) ----
            psj = psfp.tile([P, BG], f32, tag="psf", name="psj")
            nc.tensor.matmul(psj[:], lhsT=mblkT_t, rhs=beta[:], start=True, stop=True)
            zt = apool.tile([P, BG], bf16, tag="alpha", name="zt")
            nc.vector.tensor_tensor(zt[:], psj[:], alpha[:], Op.mult)

            gs = psxp.tile([G, BG], f32, tag="gs", name="gs")
            nc.tensor.matmul(gs[:], lhsT=bones4_t[:], rhs=zt[:], start=True, stop=True)
            lngs_t = cpool.tile([G, BG], f32)
            nc.scalar.activation(lngs_t[:], gs[:], Af.Ln)
            colsum_t = cpool.tile([G, 1], f32)
            nc.vector.reduce_sum(colsum_t[:], lngs_t[:], axis=AX)

            fin = psxp.tile([1, 1], f32, tag="fin", name="fin")
            nc.tensor.matmul(fin[:], lhsT=onesG_t[:], rhs=colsum_t[:], start=True, stop=True)
            outsb = cpool.tile([1, 1], f32)
            nc.vector.tensor_copy(outsb[:], fin[:])
            nc.sync.dma_start(out=out.ap(), in_=outsb[:])

    nc.compile()
    return nc


def _get_graph():
    global _GRAPH
    if _GRAPH is None:
        _GRAPH = _build_graph()
    return _GRAPH


def _host_inputs(transitions, start_transitions, end_transitions):
    """Constant / parameter-layout tensors shared by all cores (already
    exponentiated so the device preamble is DMA-only)."""
    Tm = np.asarray(transitions, np.float32)
    sv = np.asarray(start_transitions, np.float32)
    ev = np.asarray(end_transitions, np.float32)

    Mexp = np.exp(Tm).astype(BF16)
    MexpT = np.exp(Tm.T).astype(BF16)
    mblks = np.zeros((P, 2 * P), BF16)
    for g in range(G):
        sl = slice(g * 32, (g + 1) * 32)
        mblks[sl, sl] = Mexp
        mblks[sl, P + g * 32 : P + (g + 1) * 32] = MexpT

    esee = np.stack(
        [np.exp(np.tile(sv, G)), np.exp(np.tile(ev, G))], axis=1
    ).astype(np.float32)

    k = np.arange(P)
    bones4 = (np.arange(G)[None, :] == (k[:, None] // 32)).astype(BF16)  # [P, G]

    return {
        "mblks": mblks,
        "esee": np.ascontiguousarray(esee),
        "bones4": np.ascontiguousarray(bones4),
    }


_CHUNK_STEP_ORDER = None


def _chunk_step_order():
    """Scan-axis permutation: per chunk, fwd steps then (reversed) bwd steps."""
    global _CHUNK_STEP_ORDER
    if _CHUNK_STEP_ORDER is None:
        order = []
        for c in range(NCH):
            lo, n = CO[c], CS[c]
            order += list(range(lo, lo + n))                    # fwd s
            order += list(range(HALF + lo, HALF + lo + n))      # bwd slot
        _CHUNK_STEP_ORDER = np.array(order)
    return _CHUNK_STEP_ORDER


def _shard_inputs(emissions, core):
    """Per-core scan-layout emissions: per chunk, fwd steps then reversed bwd
    steps, so one contiguous DMA feeds both chains."""
    bsl = slice(core * BS, (core + 1) * BS)
    em4 = np.asarray(emissions[bsl], np.float32).reshape(G, BG, S, T)
    emf = em4[:, :, :HALF, :]                       # s = 0..255
    emb = em4[:, :, HALF:, :][:, :, ::-1, :]        # s = 511..256
    both = np.concatenate([emf, emb], axis=2)       # [G, BG, S, T]
    both = both[:, :, _chunk_step_order(), :]       # chunk-interleaved
    em_scan = both.transpose(0, 3, 2, 1).reshape(P, S * BG).astype(BF16)
    return {"em_scan": np.ascontiguousarray(em_scan)}


def _gold_host(emissions, tags, transitions, start_transitions, end_transitions):
    """Gold path score summed over the batch in float64 (pure tag indexing
    plus one emission gather)."""
    tg = np.asarray(tags).astype(np.int64)
    em = np.asarray(emissions)
    emit_sum = np.take_along_axis(em, tg[:, :, None], axis=2)[..., 0].sum(
        dtype=np.float64
    )
    trans_sum = np.asarray(transitions)[tg[:, :-1], tg[:, 1:]].sum(dtype=np.float64)
    start_sum = np.asarray(start_transitions)[tg[:, 0]].sum(dtype=np.float64)
    end_sum = np.asarray(end_transitions)[tg[:, -1]].sum(dtype=np.float64)
    return emit_sum + trans_sum + start_sum + end_sum


def _numpy_reference(emissions, tags, mask, transitions, start_transitions, end_transitions):
    """Slow numpy fallback, only used if mask is not all ones."""
    em = np.asarray(emissions, np.float64)
    tg = np.asarray(tags).astype(np.int64)
    mk = np.asarray(mask).astype(bool)
    Tm = np.asarray(transitions, np.float64)
    sv = np.asarray(start_transitions, np.float64)
    ev = np.asarray(end_transitions, np.float64)
    Bn, Sn, Tn = em.shape

    t0 = tg[:, 0]
    score = sv[t0] + np.take_along_axis(em[:, 0], t0[:, None], axis=1)[:, 0]
    maskf = mk[:, 1:].astype(np.float64)
    trans_sc = Tm[tg[:, :-1], tg[:, 1:]]
    emit_sc = np.take_along_axis(em[:, 1:], tg[:, 1:, None], axis=2)[..., 0]
    gold = score + ((trans_sc + emit_sc) * maskf).sum(axis=1)
    last_idx = mk.sum(axis=1).astype(np.int64) - 1
    last_tags = np.take_along_axis(tg, last_idx[:, None], axis=1)[:, 0]
    gold = gold + ev[last_tags]

    sc = sv[None, :] + em[:, 0]
    for s in range(1, Sn):
        nxt = sc[:, :, None] + Tm[None] + em[:, s][:, None, :]
        m = nxt.max(axis=1)
        nxt = m + np.log(np.exp(nxt - m[:, None, :]).sum(axis=1))
        sc = np.where(mk[:, s][:, None], nxt, sc)
    sc = sc + ev[None, :]
    m = sc.max(axis=1)
    fwd = m + np.log(np.exp(sc - m[:, None]).sum(axis=1))
    return np.array((fwd - gold).mean(), np.float32)


def kernel(emissions, tags, mask, transitions, start_transitions, end_transitions,
           _want_results=False, _trace=False):
    emissions = np.asarray(emissions)
    tags = np.asarray(tags)
    mask = np.asarray(mask)

    if not mask.all():
        return _numpy_reference(
            emissions, tags, mask, transitions, start_transitions, end_transitions
        )

    from concourse.bass_utils import run_bass_kernel_spmd

    nc = _get_graph()
    shared = _host_inputs(transitions, start_transitions, end_transitions)
    in_maps = []
    for c in range(NCORES):
        m = dict(shared)
        m.update(_shard_inputs(emissions, c))
        in_maps.append(m)

    res = run_bass_kernel_spmd(nc, in_maps, list(range(NCORES)), trace=_trace)

    gold = _gold_host(emissions, tags, transitions, start_transitions, end_transitions)
    tot_fwd = 0.0
    for c in range(NCORES):
        tot_fwd += float(np.asarray(res.results[c]["out"], np.float64)[0, 0])
    tot_fwd += B * S * MU
    loss = (tot_fwd - gold) / B
    if _want_results:
        return np.array(loss, np.float32), res
    return np.array(loss, np.float32)
